# revision 2
# baseline (speedup 1.0000x reference)
"""Binary segmentation loss (dice + boundary + focal) on 8 Trainium2 cores.

Data parallel: image i -> core i. Each core computes partial sums
(inter, union, focal, bsum_fg, bsum_bg) over its image; the host combines
them into the 4 scalar outputs.

v2 redesign vs baseline:
- Single ACT function table (natural_log_exp_and_others) for the whole
  kernel: sigmoid(x) = exp(-ln(1+exp(-x))), sqrt(d2) = exp(0.5*ln(d2)).
  Eliminates all steady-state LoadActFuncSet (1283 ns each; baseline
  paid 3-4 per iteration, partly on the critical path).
- Stage-2 y-window radius 2 instead of 3 (guard bound (WIN+1)^2 = 9;
  the exact max windowed d2 for these inputs is 9).
- EPS-clip of sigmoid dropped (binds only for |x| > 13.8; host guard
  falls back if max|x| ever gets close).
- sum_p accumulator dropped (union = sum(p+t) accumulated directly).
- Loop-invariant memsets/identity hoisted out of the timing loop.
- Focal tail (at, w^2, w^2*ln(pt)) on Pool in bf16; final boundary
  multiplies in bf16 (2x DVE mode).
"""

import numpy as np

H = 256
P = 128
HB = 2          # row halves: y = h*128 + p
WIN = 2         # y-window radius for stage 2
PAD = 16        # y-pad in transposed layout (keeps even/4B-aligned base)
BIG = 256.0     # "no pixel" sentinel (exact in bf16)
SEG = H + 2     # scan segment: [reset][256 cols][reset]
EPS = 1e-6
FOCAL_ALPHA = 0.25
INF = 1e10
MAX_D2_OK = (WIN + 1) ** 2  # windowed stage-2 exact iff result <= this
TINY = 2.0 ** -40  # ln bias so ln(0+TINY) stays finite; exact no-op for d2>=1

_RUNNER = None


def _build_nc(loop_reps=None, unroll=2, flat=1):
    import concourse.bacc as bacc
    import concourse.mybir as mybir
    import concourse.tile as tile

    dt = mybir.dt
    Alu = mybir.AluOpType
    Act = mybir.ActivationFunctionType

    from concourse import masks
    from concourse.hw_specs import get_activation_tables

    nc = bacc.Bacc("TRN2", target_bir_lowering=False, debug=False, num_devices=8)
    pred = nc.dram_tensor("pred", [H, H], dt.float32, kind="ExternalInput")
    targ = nc.dram_tensor("targ", [H, H], dt.float32, kind="ExternalInput")
    stats_out = nc.dram_tensor("stats", [P, 8], dt.float32, kind="ExternalOutput")

    with tile.TileContext(nc) as tc:
        with (
            tc.tile_pool(name="main", bufs=1) as pool,
            tc.tile_pool(name="tmp", bufs=4) as tmp_pool,
            tc.tile_pool(name="psum", bufs=1, space="PSUM") as psum_pool,
        ):
            SM = HB * SEG  # per-mask scan length
            NB = 2  # tile parities (double buffering across iterations)

            def dbl(shape, dtype, tag):
                return [pool.tile(shape, dtype, tag=f"{tag}{i}",
                                  name=f"{tag}{i}")
                        for i in range(NB)]

            # ---------- per-iteration tiles, double-buffered ----------
            tin = dbl([P, HB, H], dt.float32, "tin")
            xin = dbl([P, HB, H], dt.float32, "xin")
            stats = dbl([P, 8], dt.float32, "stats")
            G = dbl([P, 2, SM], dt.bfloat16, "g")
            F = dbl([P, 2, SM], dt.bfloat16, "f")
            M = dbl([P, 2, SM], dt.bfloat16, "m")
            S1T = dbl([P, 2, HB, H + 2 * PAD], dt.bfloat16, "s1t")
            D2T = dbl([P, 2, HB, H], dt.bfloat16, "d2t")
            E = dbl([P, HB, H], dt.float32, "e")
            S = dbl([P, HB, H], dt.float32, "s")
            Pt = dbl([P, HB, H], dt.float32, "pt")
            Ptb = dbl([P, HB, H], dt.bfloat16, "ptb")
            A = dbl([P, HB, H], dt.float32, "a")
            V = dbl([P, HB, H], dt.float32, "v")
            W = dbl([P, HB, H], dt.float32, "w")
            LNPT = dbl([P, HB, H], dt.float32, "lnpt")
            AT = dbl([P, HB, H], dt.float32, "at")
            SQ = dbl([P, HB, H], dt.float32, "sq")
            F1 = dbl([P, HB, H], dt.float32, "f1")
            FOCt = dbl([P, HB, H], dt.float32, "foct")
            FOCc = dbl([P, HB, H], dt.float32, "focc")
            DD = dbl([P, 2, HB, H], dt.float32, "dd")

            PS1 = [
                [psum_pool.tile([P, HB, H], dt.bfloat16, tag=f"ps1{m}{i}",
                                name=f"ps1{m}{i}")
                 for m in range(2)]
                for i in range(NB)
            ]
            PtT = [psum_pool.tile([P, HB, H], dt.bfloat16, tag=f"ptt{i}",
                                  name=f"ptt{i}")
                   for i in range(NB)]

            # ---------- one-time constants ----------
            ONES = pool.tile([P, SM], dt.bfloat16)
            ident = pool.tile([P, P], dt.bfloat16)
            TINYC = pool.tile([P, 1], dt.float32)
            Ovs = ONES[:].rearrange("p (h x) -> p h x", h=HB)
            nc.gpsimd.memset(ONES[:], 1.0)
            nc.gpsimd.memset(Ovs[:, :, 0:1], BIG)
            nc.gpsimd.memset(Ovs[:, :, SEG - 1 : SEG], BIG)
            masks.make_identity(nc, ident[:])
            nc.gpsimd.memset(TINYC[:], TINY)
            for i in range(NB):
                nc.gpsimd.memset(S1T[i][:, :, :, 0:PAD], BIG)
                nc.gpsimd.memset(S1T[i][:, :, :, PAD + H :], BIG)
                for m in range(2):
                    Gmv = G[i][:, m].rearrange("p (h x) -> p h x", h=HB)
                    nc.gpsimd.memset(Gmv[:, :, 0:1], BIG)
                    nc.gpsimd.memset(Gmv[:, :, SEG - 1 : SEG], BIG)

            # natural_log_exp_and_others covers exp/ln/square/copy — the
            # whole kernel. One explicit preload; the table-load pass then
            # inserts no implicit (1283 ns) loads.
            nle_id = list(get_activation_tables(nc.m.arch)).index(
                "natural_log_exp_and_others"
            )
            nc.scalar.add_instruction(mybir.InstLoadActFuncSet(
                name=nc.get_next_instruction_name(), act_func_set_id=nle_id,
                ins=[], outs=[],
            ))

            def body(i):
                # ---- load inputs; targ first (it gates the EDT chain) ----
                nc.sync.dma_start(
                    tin[i][:], targ.ap().rearrange("(h p) x -> p h x", h=HB)
                )
                nc.scalar.dma_start(
                    xin[i][:], pred.ap().rearrange("(h p) x -> p h x", h=HB)
                )

                # ---- sigmoid via exp/ln/exp ----
                nc.scalar.activation(E[i][:], xin[i][:], Act.Exp, scale=-1.0)
                nc.scalar.activation(S[i][:], E[i][:], Act.Ln, bias=1.0)
                pt_inst = nc.scalar.activation(
                    Pt[i][:], S[i][:], Act.Exp, scale=-1.0
                )

                # ---- EDT stage 1: row masks + fwd/bwd scans (DVE),
                # PE transpose, square into padded SBUF (ACT) ----
                for m in range(2):
                    Gm = G[i][:, m]
                    Gmv = Gm.rearrange("p (h x) -> p h x", h=HB)
                    for h in range(HB):
                        nc.vector.tensor_scalar(
                            Gmv[:, h, 1 : 1 + H], tin[i][:, h], 0.5, BIG,
                            op0=(Alu.is_le if m == 0 else Alu.is_gt),
                            op1=Alu.mult,
                        )
                    nc.vector.tensor_tensor_scan(
                        F[i][:, m], ONES[:], Gm, BIG, op0=Alu.add, op1=Alu.min
                    )
                    nc.vector.tensor_tensor_scan(
                        M[i][:, m, ::-1], ONES[:, ::-1], F[i][:, m, ::-1],
                        BIG, op0=Alu.add, op1=Alu.min,
                    )
                    Mmv = M[i][:, m].rearrange("p (h x) -> p h x", h=HB)
                    for g in range(HB):
                        for h in range(HB):
                            nc.tensor.transpose(
                                PS1[i][m][:, g, P * h : P * h + P],
                                Mmv[:, h, 1 + P * g : 1 + P * g + P],
                                ident[:],
                            )
                    sq_inst = nc.scalar.activation(
                        S1T[i][:, m, :, PAD : PAD + H], PS1[i][m][:],
                        Act.Square,
                    )
                    if m == 0:
                        tile.add_dep_helper(
                            sq_inst.ins, pt_inst.ins, sync=False,
                            reason="keep sigma ahead of square copies on ACT",
                        )

                # sigma^2 (bf16) + PE-transposed copy: the bsum tail is
                # d*sigma = exp(0.5*ln(d2*sigma^2)), so the final
                # accumulation runs on ACT and DVE only pays a bf16 mult
                nc.gpsimd.tensor_tensor(
                    Ptb[i][:], Pt[i][:], Pt[i][:], op=Alu.mult
                )
                for g in range(HB):
                    for h in range(HB):
                        nc.tensor.transpose(
                            PtT[i][:, g, P * h : P * h + P],
                            Ptb[i][:, h, P * g : P * g + P],
                            ident[:],
                        )

                # ---- dice/focal elementwise (fills the DVE gap between
                # the scans and stage 2) ----
                nc.vector.scalar_tensor_tensor(
                    A[i][:], Pt[i][:], 1.0, tin[i][:], op0=Alu.mult,
                    op1=Alu.mult, accum_out=stats[i][:, 0:1],
                )
                nc.vector.scalar_tensor_tensor(
                    V[i][:], Pt[i][:], 1.0, tin[i][:], op0=Alu.mult,
                    op1=Alu.add, accum_out=stats[i][:, 1:2],
                )
                w_inst = nc.vector.scalar_tensor_tensor(
                    W[i][:], A[i][:], 2.0, V[i][:], op0=Alu.mult,
                    op1=Alu.subtract,
                )
                nc.scalar.activation(LNPT[i][:], W[i][:], Act.Ln, bias=1.0)
                nc.gpsimd.tensor_scalar(
                    AT[i][:], tin[i][:], -0.5, 0.75, op0=Alu.mult, op1=Alu.add
                )
                nc.scalar.activation(SQ[i][:], W[i][:], Act.Square)
                nc.gpsimd.tensor_tensor(
                    F1[i][:], SQ[i][:], LNPT[i][:], op=Alu.mult
                )

                # ---- stage 2 (window +-2 in y): d2 = min(s1,
                # min(u<<1,u>>1), min(v<<2,v>>2)), u = s1+1, v = s1+4 ----
                for m in range(2):
                    C = S1T[i][:, m, :, PAD : PAD + H]
                    U = tmp_pool.tile(
                        [P, HB, H + 2 * PAD], dt.bfloat16, tag="u"
                    )
                    u_inst = nc.vector.tensor_scalar(
                        U[:], S1T[i][:, m], 1.0, None, op0=Alu.add
                    )
                    if m == 0:
                        tile.add_dep_helper(
                            u_inst.ins, w_inst.ins, sync=False,
                            reason="let A/V/W fill the DVE gap before stage 2",
                        )
                    Vv = tmp_pool.tile(
                        [P, HB, H + 2 * PAD], dt.bfloat16, tag="v"
                    )
                    nc.vector.tensor_scalar(
                        Vv[:], S1T[i][:, m], 4.0, None, op0=Alu.add
                    )
                    M1 = tmp_pool.tile([P, HB, H], dt.bfloat16, tag="m1")
                    nc.vector.tensor_tensor(
                        M1[:], U[:, :, PAD - 1 : PAD - 1 + H],
                        U[:, :, PAD + 1 : PAD + 1 + H], op=Alu.min,
                    )
                    M2 = tmp_pool.tile([P, HB, H], dt.bfloat16, tag="m2")
                    nc.vector.tensor_tensor(
                        M2[:], Vv[:, :, PAD - 2 : PAD - 2 + H],
                        Vv[:, :, PAD + 2 : PAD + 2 + H], op=Alu.min,
                    )
                    X = tmp_pool.tile([P, HB, H], dt.bfloat16, tag="x")
                    nc.vector.tensor_tensor(X[:], M1[:], M2[:], op=Alu.min)
                    nc.vector.tensor_tensor(D2T[i][:, m], X[:], C, op=Alu.min)

                # ---- bsum tail, then focal tail (ACT order matters:
                # ln/exp-accum pairs first, focal copy-accum last) ----
                for m in range(2):
                    DSQ = tmp_pool.tile([P, HB, H], dt.bfloat16, tag="dsq")
                    nc.vector.tensor_tensor(
                        DSQ[:], D2T[i][:, m], PtT[i][:], op=Alu.mult
                    )
                    LNDm = tmp_pool.tile([P, HB, H], dt.float32, tag="lnd")
                    nc.scalar.activation(
                        LNDm[:], DSQ[:], Act.Ln, bias=TINYC[:]
                    )
                    nc.scalar.activation(
                        DD[i][:, m], LNDm[:], Act.Exp, scale=0.5,
                        accum_out=stats[i][:, 3 + m : 4 + m],
                    )

                # col2 = sum(at * w^2 * ln(pt)); host negates
                nc.gpsimd.tensor_tensor(
                    FOCt[i][:], AT[i][:], F1[i][:], op=Alu.mult
                )
                nc.scalar.activation(
                    FOCc[i][:], FOCt[i][:], Act.Copy,
                    accum_out=stats[i][:, 2:3],
                )

                nc.sync.dma_start(stats_out.ap()[:, 0:5], stats[i][:, 0:5])

            if loop_reps:
                assert loop_reps % unroll == 0
                with tc.For_i(0, loop_reps // unroll, 1):
                    for u in range(unroll):
                        body(u % NB)
            else:
                for u in range(flat):
                    body(u % NB)

    nc.compile()
    return nc


def _get_runner(loop_reps=None):
    """Build the Bass program + jitted PJRT executable once; return a
    callable (pred8, targ8) -> stats [8, 128, 8]."""
    global _RUNNER
    if _RUNNER is None:
        _RUNNER = {}
    if loop_reps in _RUNNER:
        return _RUNNER[loop_reps]

    import jax
    import concourse.mybir as mybir
    from concourse import bass2jax
    from jax.sharding import Mesh, PartitionSpec
    from jax.experimental.shard_map import shard_map

    bass2jax.install_neuronx_cc_hook()
    unroll = 1
    if loop_reps:
        for u in (8, 4, 2, 1):
            if loop_reps % u == 0:
                unroll = u
                break
    nc = _build_nc(loop_reps, unroll=unroll)

    n_cores = 8
    partition_name = (
        nc.partition_id_tensor.name if nc.partition_id_tensor else None
    )
    in_names, out_names, out_avals, zero_outs = [], [], [], []
    for alloc in nc.m.functions[0].allocations:
        if not isinstance(alloc, mybir.MemoryLocationSet):
            continue
        name = alloc.memorylocations[0].name
        if alloc.kind == "ExternalInput":
            if name != partition_name:
                in_names.append(name)
        elif alloc.kind == "ExternalOutput":
            shape = tuple(alloc.tensor_shape)
            dtype = mybir.dt.np(alloc.dtype)
            out_names.append(name)
            out_avals.append(jax.core.ShapedArray(shape, dtype))
            zero_outs.append(np.zeros(shape, dtype))
    n_params = len(in_names)
    all_names = in_names + out_names
    if partition_name is not None:
        all_names.append(partition_name)

    def _body(*args):
        operands = list(args)
        if partition_name is not None:
            operands.append(bass2jax.partition_id_tensor())
        outs = bass2jax._bass_exec_p.bind(
            *operands,
            out_avals=tuple(out_avals),
            in_names=tuple(all_names),
            out_names=tuple(out_names),
            lowering_input_output_aliases=(),
            sim_require_finite=True,
            sim_require_nnan=True,
            nc=nc,
        )
        return tuple(outs)

    devices = jax.devices()[:n_cores]
    mesh = Mesh(np.asarray(devices), ("core",))
    n_ops = n_params + len(out_names)
    sharded = jax.jit(
        shard_map(
            _body,
            mesh=mesh,
            in_specs=(PartitionSpec("core"),) * n_ops,
            out_specs=(PartitionSpec("core"),) * len(out_names),
            check_rep=False,
        ),
        donate_argnums=tuple(range(n_params, n_ops)),
        keep_unused=True,
    )
    concat_zero_shapes = [
        ((n_cores * z.shape[0],) + z.shape[1:], z.dtype) for z in zero_outs
    ]

    def run(pred8, targ8):
        ins = {"pred": pred8, "targ": targ8}
        concat_in = [
            np.ascontiguousarray(ins[name]).reshape(n_cores * H, H)
            for name in in_names
        ]
        zeros = [np.zeros(s, d) for s, d in concat_zero_shapes]
        out_arrs = sharded(*concat_in, *zeros)
        st = np.asarray(out_arrs[0])
        return st.reshape(n_cores, P, 8)

    _RUNNER[loop_reps] = run
    return run


# ---------------- host-side exact fallback (near-never path) ----------------

def _np_row_dist(mask):
    """Per-row 1D L1 distance to nearest True, BIG if row empty. [H,W]"""
    Hh, Wd = mask.shape
    f = np.full((Hh,), BIG, np.float32)
    out_f = np.empty((Hh, Wd), np.float32)
    for x in range(Wd):
        f = np.minimum(f + 1.0, np.where(mask[:, x], 0.0, BIG))
        out_f[:, x] = f
    b = np.full((Hh,), BIG, np.float32)
    out_b = np.empty((Hh, Wd), np.float32)
    for x in range(Wd - 1, -1, -1):
        b = np.minimum(b + 1.0, np.where(mask[:, x], 0.0, BIG))
        out_b[:, x] = b
    return np.minimum(out_f, out_b)


def _np_win_d2(mask):
    """Windowed stage-2 result (same algorithm as the device kernel)."""
    s1 = _np_row_dist(mask) ** 2
    Hh = s1.shape[0]
    pad = np.full((WIN, s1.shape[1]), BIG * BIG, np.float32)
    s1p = np.concatenate([pad, s1, pad], axis=0)
    d2 = s1.copy()
    for d in range(1, WIN + 1):
        m = np.minimum(s1p[WIN - d : WIN - d + Hh], s1p[WIN + d : WIN + d + Hh])
        d2 = np.minimum(d2, m + d * d)
    return d2


def _np_exact_edt(mask):
    """Exact EDT matching the reference formula (incl. empty-mask fallback)."""
    Hh, Wd = mask.shape
    ax = np.arange(Wd, dtype=np.float32)
    dx2 = (ax[:, None] - ax[None, :]) ** 2
    d1 = np.where(mask[:, None, :], dx2[None, :, :], INF).min(-1)
    ay = np.arange(Hh, dtype=np.float32)
    dy2 = (ay[:, None] - ay[None, :]) ** 2
    d = (dy2[:, :, None] + d1[None, :, :]).min(1)
    max_d2 = float((Hh - 1) ** 2 + (Wd - 1) ** 2)
    d = np.where(d > INF * 0.5, max_d2, d)
    return np.sqrt(d)


def _np_boundary_sum(pred_img, targ_img):
    """Exact sum(phi * sigmoid(pred)) for one image, reference semantics."""
    fg = targ_img > 0.5
    phi = np.where(fg, -_np_exact_edt(~fg), _np_exact_edt(fg))
    p = 1.0 / (1.0 + np.exp(-pred_img.astype(np.float64)))
    return float((phi.astype(np.float64) * p).sum())


def _np_focal_dice(pred_img, targ_img):
    """Exact (inter, union, fsum) for one image, reference semantics."""
    p = 1.0 / (1.0 + np.exp(-pred_img.astype(np.float64)))
    t = targ_img.astype(np.float64)
    pc = np.clip(p, EPS, 1.0 - EPS)
    pt = pc * t + (1.0 - pc) * (1.0 - t)
    at = FOCAL_ALPHA * t + (1.0 - FOCAL_ALPHA) * (1.0 - t)
    foc = -at * (1.0 - pt) ** 2 * np.log(pt)
    return float((pc * t).sum()), float((p + t).sum()), float(foc.sum())


# ---------------------------------- entry ----------------------------------

def kernel(pred_masks, target_masks):
    pred8 = np.asarray(pred_masks, dtype=np.float32).reshape(8, H, H)
    targ8 = np.asarray(target_masks, dtype=np.float32).reshape(8, H, H)

    stats = _get_runner()(pred8, targ8)  # [8, 128, 8]
    cols = stats.astype(np.float64).sum(axis=1)  # [8, 8]
    inter = cols[:, 0]
    union = cols[:, 1]
    fsum = -cols[:, 2]
    bsum = cols[:, 3] - cols[:, 4]  # sum(d_fg*p) - sum(d_bg*p)

    n_el = float(H * H)

    # guards: stage-2 window must have been sufficient for both masks;
    # the EPS clip must not bind (it binds only for |x| > ~13.8)
    for i in range(8):
        fg = targ8[i] > 0.5
        if (not fg.any()) or fg.all() or \
           _np_win_d2(fg).max() > MAX_D2_OK or \
           _np_win_d2(~fg).max() > MAX_D2_OK:
            bsum[i] = _np_boundary_sum(pred8[i], targ8[i])
        if np.abs(pred8[i]).max() > 13.0:
            inter[i], union[i], fsum[i] = _np_focal_dice(pred8[i], targ8[i])

    ratios = (2.0 * inter + EPS) / (union + EPS)
    dice_val = 1.0 - ratios.mean()
    boundary_val = bsum.sum() / (8.0 * n_el)
    focal_val = fsum.sum() / (8.0 * n_el)
    loss = dice_val + boundary_val + focal_val
    return (
        np.float32(loss),
        np.float32(dice_val),
        np.float32(boundary_val),
        np.float32(focal_val),
    )
